# revision 19
# baseline (speedup 1.0000x reference)
"""DenseGATConv Bass/Tile kernel for Trainium2, SPMD over 8 NeuronCores.

Problem (B=4, N=2048, F=128, H=4, C=64):
  xh = (x @ W).reshape(B,N,H,C)
  a_src[b,j,h] = xh . att_src ; a_dst[b,i,h] = xh . att_dst
  s = a_src[j] + a_dst[i];  alpha = softmax_j(mask(adj+I, leaky_relu(s, 0.2)))
  out[b,i] = concat_h(sum_j alpha * xh[b,j,h,:]) + bias

Algebra (no exp over the N*N*H grid):
  exp(lrelu(s)) = Es_j * Ed_i * max(q'_i * es'_j, 1)
      Es = exp(a_src), q' = exp(-0.8 a_dst), es' = exp(-0.8 a_src)
  Ed_i cancels in the softmax ratio, so the masked grid weight is
      G[j,i] = adjT[j,i] * Es_j * T'[j,i],   T' = max(q'_i * es'_j, 1)
  with Es_j folded into the host-prepped stationary [xh*Es | Es] so the
  PE accumulates numerator rows 0:64 and the denominator in row 64:
      acc[h][c|den, i] += [xh*Es | Es]^T @ (T' * adjT)

Device work per j-tile (the N^2*H grid, split DVE/ACT to balance):
  T' pass  — per head: either DVE tensor_scalar (mult,max fused; 4x fp16
             mode) or ACT relu(es'*q - 1) with per-partition scale/bias
             followed by a 4x-mode DVE +1 tensor_scalar (N_ACT of the 32
             (tile, head-pair) units take the ACT route).
  mask op  — ONE 4-head DVE tensor_tensor (T' * adjT, fp16 2x mode),
             adjT broadcast across heads with a stride-0 axis.
  matmul   — PE: acc[h] += stationary^T @ G, fp16, f32 PSUM (8 banks);
             stationaries zero-padded to 128 cols to trigger FWL.

Host does the projections (x@W and the tiny attention dots are weight-prep
scale work), and the gather step does num/den + bias + layout transpose.
Sharding: core = b*2 + ihalf; each core owns 1024 destination rows of one
batch and reads that batch's full source side.
"""

import os

import numpy as np

import concourse.bacc as bacc
import concourse.bass as bass
import concourse.tile as tile
from concourse import mybir
from concourse.bass_utils import run_bass_kernel_spmd

B, N, F = 4, 2048, 128
H, C = 4, 64
HC = H * C
N_CORES = 8
ID = N // 2          # dest rows per core
NT = N // 128        # 16 source tiles
NKD = ID // 512      # 2 dest 512-chunks
F32 = mybir.dt.float32
F16 = mybir.dt.float16

TBUFS = int(os.environ.get('TBUFS', 5))
GBUFS = int(os.environ.get('GBUFS', 4))
ABUFS = int(os.environ.get('ABUFS', 4))
# head-pair path assignment over the 32 (tile, pair) units:
N_ACT = int(os.environ.get('N_ACT', 24))   # pairs whose T' runs on ACT
N_GP = int(os.environ.get('N_GP', 0))      # pairs whose T' runs on GPSIMD

_NC_CACHE = {}


def _pair_paths():
    """Bresenham-interleave the ACT/GPSIMD/DVE path assignment over the
    32 (tile, head-pair) units so each engine's work is spread in time."""
    n_d = 32 - N_ACT - N_GP
    # pin D (DVE-self-sufficient) units at the very start (so DVE ramps
    # while ACT warms up) and at the very end (shorter tail chain);
    # Bresenham-spread the rest so neither engine starves mid-stream.
    head = ['D'] * min(2, n_d)
    tail = ['D'] * min(1, max(0, n_d - 2))
    mid_n = 32 - len(head) - len(tail)
    cnt = {'A': N_ACT, 'G': N_GP, 'D': n_d - len(head) - len(tail)}
    err = {k: 0.0 for k in cnt}
    seq = []
    for _ in range(mid_n):
        for k in cnt:
            err[k] += cnt[k] / mid_n
        pick = max(err, key=lambda k: err[k])
        err[pick] -= 1.0
        seq.append(pick)
    return head + seq + tail


def build_nc(reps: int = 1):
    nc = bacc.Bacc("TRN2", target_bir_lowering=False, debug=False, num_devices=1)

    d_adjT = nc.dram_tensor("adjT", [NT, 128, ID], F16, kind="ExternalInput").ap()
    d_xes = nc.dram_tensor("xes", [4, 128, 4, H, 128], F16, kind="ExternalInput").ap()
    d_esp = nc.dram_tensor("esp", [128, NT, H], F32, kind="ExternalInput").ap()
    d_qbc = nc.dram_tensor("qbc", [128, H, ID], F16, kind="ExternalInput").ap()
    d_out = nc.dram_tensor("out", [H, 65, ID], F32, kind="ExternalOutput").ap()

    RELU = mybir.ActivationFunctionType.Relu
    CPY = mybir.ActivationFunctionType.Copy
    paths = _pair_paths()

    with tile.TileContext(nc) as tc:
        with tc.tile_pool(name="const", bufs=1) as const:
            neg1 = const.tile([128, 1], F32)
            nc.vector.memset(neg1, -1.0)
            # warm the ACT spline tables while the prefix DMAs run
            scratch1 = const.tile([1, 1], F32)
            nc.scalar.activation(scratch1, neg1[0:1, :], RELU,
                                 bias=neg1[0:1, :])
            q_bc = const.tile([128, H, ID], F16)
            xes = const.tile([128, NT, H, 128], F16)
            # first-tile-critical data first: heads 0/1, tile group 0
            esp = const.tile([128, NT, H], F32)
            nc.sync.dma_start(out=q_bc[:, 0, :], in_=d_qbc[:, 0, :])
            nc.scalar.dma_start(out=esp, in_=d_esp)
            nc.sync.dma_start(out=q_bc[:, 1, :], in_=d_qbc[:, 1, :])
            nc.sync.dma_start(out=q_bc[:, 2, :], in_=d_qbc[:, 2, :])
            nc.sync.dma_start(out=q_bc[:, 3, :], in_=d_qbc[:, 3, :])
            nc.scalar.dma_start(out=xes[:, 0:4], in_=d_xes[0])

            with tc.tile_pool(name="ep_sb", bufs=1) as epsb, \
                 tc.tile_pool(name="acc", bufs=1, space="PSUM") as accp:
                acc = {}
                for h in range(H):
                    acc[h] = accp.tile([128, ID], F32, tag=f"acc{h}",
                                       name=f"acc{h}")

                sc_b = nc.enter_named_scope("phB", False)
                with tc.tile_pool(name="adj", bufs=ABUFS) as adjp, \
                     tc.tile_pool(name="grid", bufs=4) as gridp:
                    for rep in range(reps):
                        for t in range(NT):
                            adjt = adjp.tile([128, ID], F16)
                            nc.sync.dma_start(out=adjt, in_=d_adjT[t])
                            if rep == 0 and t == 1:
                                for gg in range(1, 4):
                                    nc.scalar.dma_start(
                                        out=xes[:, 4 * gg:4 * (gg + 1)],
                                        in_=d_xes[gg])
                            first = (rep == 0 and t == 0)
                            last = (rep == reps - 1 and t == NT - 1)
                            tp = gridp.tile([128, H, ID], F16, tag="T",
                                            bufs=TBUFS)
                            for pair in range(2):
                                h0 = 2 * pair
                                path = paths[(t * 2 + pair) % 32]
                                if path in ('A', 'G'):
                                    # R = relu(q*es' - 1), then T' = R + 1 on
                                    # a cheap 4x-mode 2-head tensor_scalar
                                    for hh in range(2):
                                        nc.scalar.activation(
                                            tp[:, h0 + hh, :], q_bc[:, h0 + hh, :],
                                            RELU, bias=neg1,
                                            scale=esp[:, t, h0 + hh:h0 + hh + 1])
                                    eng = nc.vector if path == 'A' else nc.gpsimd
                                    eng.tensor_scalar(
                                        out=tp[:, h0:h0 + 2, :],
                                        in0=tp[:, h0:h0 + 2, :], scalar1=1.0,
                                        scalar2=None,
                                        op0=mybir.AluOpType.add)
                                else:
                                    for hh in range(2):
                                        nc.vector.tensor_scalar(
                                            out=tp[:, h0 + hh, :],
                                            in0=q_bc[:, h0 + hh, :],
                                            scalar1=esp[:, t, h0 + hh:h0 + hh + 1],
                                            scalar2=1.0,
                                            op0=mybir.AluOpType.mult,
                                            op1=mybir.AluOpType.max)
                            g = gridp.tile([128, H, ID], F16, tag="G",
                                           bufs=GBUFS)
                            if rep == 0 and t < 2:
                                # ramp: per-pair mask op so pair 0 streams as
                                # soon as q_bc heads 0/1 land (head 3's DMA
                                # arrives ~5us later than head 0's)
                                adj_rep2 = bass.AP(
                                    tensor=adjt.tensor, offset=adjt.offset,
                                    ap=[adjt.ap[0], [0, 2]] + list(adjt.ap[1:]))
                                for pair in range(2):
                                    h0 = 2 * pair
                                    nc.vector.tensor_tensor(
                                        out=g[:, h0:h0 + 2, :],
                                        in0=tp[:, h0:h0 + 2, :], in1=adj_rep2,
                                        op=mybir.AluOpType.mult)
                                    for hh in range(2):
                                        for k in range(NKD):
                                            nc.tensor.matmul(
                                                acc[h0 + hh][:, k * 512:(k + 1) * 512],
                                                xes[:, t, h0 + hh, :],
                                                g[:, h0 + hh, k * 512:(k + 1) * 512],
                                                start=first, stop=last)
                            else:
                                adj_rep4 = bass.AP(
                                    tensor=adjt.tensor, offset=adjt.offset,
                                    ap=[adjt.ap[0], [0, H]] + list(adjt.ap[1:]))
                                nc.vector.tensor_tensor(
                                    out=g, in0=tp, in1=adj_rep4,
                                    op=mybir.AluOpType.mult)
                                for h in range(H):
                                    for k in range(NKD):
                                        nc.tensor.matmul(
                                            acc[h][:, k * 512:(k + 1) * 512],
                                            xes[:, t, h, :],  # 128-col stationary (FWL)
                                            g[:, h, k * 512:(k + 1) * 512],
                                            start=first, stop=last)

                nc.leave_named_scope("phB", sc_b[0], False)
                sc_c = nc.enter_named_scope("phC", False)
                # evacuate accumulators and ship raw num|den rows; the host
                # gather does num/den + bias + transpose
                for h in range(H):
                    s = epsb.tile([65, ID], F32, tag=f"s{h}", name=f"s{h}")
                    nc.scalar.activation(s, acc[h][0:65, :], CPY)
                    if h % 2 == 0:
                        nc.sync.dma_start(out=d_out[h], in_=s)
                    else:
                        nc.scalar.dma_start(out=d_out[h], in_=s)
                nc.leave_named_scope("phC", sc_c[0], False)

    nc.compile()
    return nc


def _get_nc(reps: int = 1):
    if reps not in _NC_CACHE:
        _NC_CACHE[reps] = build_nc(reps)
    return _NC_CACHE[reps]


def make_in_maps(x, adj, W, att_src, att_dst, bias):
    x = np.asarray(x, dtype=np.float32)
    adj = np.asarray(adj, dtype=np.float32)
    W = np.asarray(W, dtype=np.float32)
    att_src = np.asarray(att_src, dtype=np.float32)
    att_dst = np.asarray(att_dst, dtype=np.float32)

    xh = (x.reshape(B * N, F) @ W).reshape(B, N, H, C)
    a_src = np.einsum('bnhc,hc->bnh', xh, att_src)
    a_dst = np.einsum('bnhc,hc->bnh', xh, att_dst)
    # shift by the per-(b,h) max: scales num and den equally (softmax-
    # invariant) and keeps Es / xh*Es within fp16 range for any input scale
    Es = np.exp(a_src - a_src.max(axis=1, keepdims=True))   # [B, N, H]
    esp = np.exp(-0.8 * a_src)              # [B, N, H]
    # clamp below fp16 max so out-of-range rows saturate instead of
    # producing Inf (Inf * 0-mask would poison PSUM with NaN)
    qp = np.minimum(np.exp(-0.8 * a_dst), 6.0e4)    # [B, N, H]

    adjl = adj.copy()
    idx = np.arange(N)
    adjl[:, idx, idx] = 1.0

    # stationaries [xh*Es | Es] per head, partition-major per 128-row tile
    xes = np.zeros((B, N, H, 128), dtype=np.float16)
    xes[..., 0:64] = xh * Es[..., None]
    xes[..., 64] = Es

    in_maps = []
    for c in range(N_CORES):
        b, half = c // 2, c % 2
        adjT = np.ascontiguousarray(
            adjl[b].T[:, half * ID:(half + 1) * ID]).astype(np.float16)
        qbc = np.broadcast_to(
            qp[b, half * ID:(half + 1) * ID, :].T.astype(np.float16)[None],
            (128, H, ID))
        in_maps.append({
            "adjT": adjT.reshape(NT, 128, ID),
            "xes": np.ascontiguousarray(
                xes[b].reshape(4, 4, 128, H, 128).transpose(0, 2, 1, 3, 4)),
            "esp": np.ascontiguousarray(
                esp[b].reshape(NT, 128, H).transpose(1, 0, 2)),
            "qbc": np.ascontiguousarray(qbc),
        })
    return in_maps, np.asarray(bias, dtype=np.float32)


def assemble(results, bias):
    out = np.empty((B, N, HC), dtype=np.float32)
    for c in range(N_CORES):
        b, half = c // 2, c % 2
        r = results[c]["out"]               # [H, 65, ID]
        num = r[:, 0:64, :]                 # [H, 64, ID]
        den = r[:, 64, :]                   # [H, ID]
        o = (num / den[:, None, :]).transpose(2, 0, 1).reshape(ID, HC)
        out[b, half * ID:(half + 1) * ID, :] = o
    return out + bias


def kernel(x, adj, W, att_src, att_dst, bias):
    nc = _get_nc(1)
    in_maps, bias_v = make_in_maps(x, adj, W, att_src, att_dst, bias)
    res = run_bass_kernel_spmd(nc, in_maps, list(range(N_CORES)))
    return assemble(res.results, bias_v)
